# revision 40
# baseline (speedup 1.0000x reference)
"""LISSOM cortex layer forward pass on 8 Trainium2 NeuronCores.

Math (reference):
    afferent = clamp(x @ Wr, 0, 1)                      # [B, N]
    exc      = clamp(afferent @ We, 0, 1)               # [B, N]
    inh      = clamp(afferent @ Wi, 0, 1)               # [B, N]
    out      = clamp(afferent + 0.2*exc - 0.4*inh, 0, 1)

Structural facts exploited:
  * All weight columns are nonnegative with L1 norm exactly 1 and
    x in [0,1), so afferent/exc/inh are convex averages in [0,1): the
    three inner clamps never bind, and the final pre-activation lies in
    [0.38, 0.42] so the outer clamp never binds either.  With
    a' = afferent - 0.5 (column sums are exactly 1):
        out = a' @ (I + 0.2 We)  +  a' @ (-0.4 Wi)  +  0.4
  * We has a radius-2 circular mask: 13 nonzero diagonals with offsets
    in [-192, 192].  (I + 0.2 We) is fed as a banded set of
    [7*128, 384] blocks - a 7-chunk afferent window per 384-column
    output slice - instead of a dense [N, N/8] matrix.
  * Centering: x8 = (x-0.5)*s_x and a8 = a'*s_a in fp8 e4m3 make the
    quantization error proportional to the small deviations (~1e-2)
    instead of the 0.5-level magnitudes.
  * All big matmul streams are fp8 with perf_mode=DoubleRow (two
    128-row k-chunks per instruction at 0.5 cycles/row).

Sharding: weight columns split across 8 cores.  Each core computes its
afferent slice in two column pieces (512 + 640), transposes each on
the PE (bf16) with an Activation-engine fp8 convert, and TWO pipelined
fp8 AllGathers distribute the full centered afferent^T: the first
(4/9 of the slice) launches while the rest of the retina stream is
still loading; the second overlaps the inhibitory weight stream.  The
gather payload is partition-chunked [128, chunks*32] so the unpack is
a cheap 96/192-byte-cell gather, and the band halo is two tiny
rectangles read with partition_id()-based dynamic offsets (ring
neighbors, SPMD-uniform).  Band matmuls on local chunks run during the
gathers; the inhibitory matmuls consume unpacked halves as they land.
"""

import sys

if "/opt/trn_rl_repo" not in sys.path:
    sys.path.insert(0, "/opt/trn_rl_repo")

import ml_dtypes
import numpy as np

import concourse.bass as bass
import concourse.bacc as bacc
import concourse.mybir as mybir
import concourse.tile as tile
from concourse.ap import AP
from concourse.tile import add_dep_helper
from concourse.bass_utils import run_bass_kernel_spmd

B = 32            # batch
N = 9216          # neurons
CORES = 8
S = N // CORES    # 1152 columns per core
KP = 128          # contraction tile (partition dim)
KC = N // KP      # 72 k-chunks
PAIRS = KC // 2   # 36 DoubleRow pair-chunks
NS = 384          # matmul free-dim slice (1 PSUM bank each)
NJ = S // NS      # 3 n-slices
WBLK = 8          # k-chunks per weight DMA block
NBLK = KC // WBLK  # 9 blocks per weight stream
BANDC = 7         # afferent chunks per band j-block (384 + 2*192 rows)
MC = S // KP      # 9 local afferent chunks per core
M1 = 4            # chunks in gather half 1
M2 = MC - M1      # chunks in gather half 2
NSA = M1 * KP     # 512 afferent columns in piece A
WIBBLK = 6        # k-chunks per wib DMA block
NWIB = KC // WIBBLK

S_X = 256.0       # fp8 scale for centered x
S_A = 2048.0      # fp8 scale for centered afferent
S_B = 192.0       # fp8 scale for the banded (I + 0.2 We)

F32 = mybir.dt.float32
BF16 = mybir.dt.bfloat16
E8 = mybir.dt.float8e4  # e4m3

# band matmul plan: per j-slice the window positions are [3j, 3j+7) in
# the 13-chunk window; positions [2, 11) are the core's own chunks
# (available pre-AllGather from the local transpose).  Entries:
# (rel_block, n_chunks, position, local)
def _band_plan():
    plans = []
    for j in range(NJ):
        ops = []
        for rel in (0, 2, 4):
            p = 3 * j + rel
            if p >= 2 and p + 1 < 11:
                ops.append((rel, 2, p, True))
            elif p + 1 < 2 or p >= 11:
                ops.append((rel, 2, p, False))
            else:  # pair straddles the local/halo boundary: two singles
                for q in range(2):
                    ops.append((rel + q, 1, p + q, 2 <= p + q < 11))
        p = 3 * j + 6
        ops.append((6, 1, p, 2 <= p < 11))
        plans.append(ops)
    return plans


BAND_PLAN = _band_plan()

# inhibitory pair schedule: pairs whose both chunks have m = g%9 < M1
# are servable from the first gather half (they run during AG2)
PASS1 = [t2 for t2 in range(PAIRS)
         if (2 * t2) % MC < M1 and (2 * t2 + 1) % MC < M1
         and (2 * t2 + 1) < (NWIB - 1) * WIBBLK]
PASS2 = [t2 for t2 in range(PAIRS) if t2 not in PASS1]


def build_nc():
    np_bf = ml_dtypes.bfloat16

    nc = bacc.Bacc("TRN2", num_devices=CORES)

    xT_d = nc.dram_tensor("xT", [KP, KC * B], E8, kind="ExternalInput")
    # retina weights pre-transposed to partition-major, split by columns
    wrA_d = nc.dram_tensor("wrA", [KP, KC * NSA], E8, kind="ExternalInput")
    wrB_d = nc.dram_tensor("wrB", [KP, KC * (S - NSA)], E8,
                           kind="ExternalInput")
    wib_d = nc.dram_tensor("wib", [N, S], E8, kind="ExternalInput")
    # partition-major band layout: band_d[p, t*NS+s] = band block t, row p
    band_d = nc.dram_tensor("band", [KP, NJ * BANDC * NS], E8,
                            kind="ExternalInput")
    # per-partition scalars: [s_a/(s_x*sr), 1/(s_a*si), 1/(s_a*s_b)]
    scales_d = nc.dram_tensor("scales", [B, 3], F32, kind="ExternalInput")
    out_d = nc.dram_tensor("out", [B, S], F32, kind="ExternalOutput")
    ident_d = nc.inline_tensor(np.eye(32, dtype=np_bf), name="ident32")

    rg = [list(range(CORES))]
    DR = mybir.MatmulPerfMode.DoubleRow

    with tile.TileContext(nc) as tc:
        with (
            tc.tile_pool(name="persist", bufs=1) as persist,
            tc.tile_pool(name="wr", bufs=5) as wrp,
            tc.tile_pool(name="ps", bufs=1, space="PSUM") as ps,
            tc.tile_pool(name="dram", bufs=1, space="DRAM") as dram,
        ):
            pid = nc.partition_id(engines=[mybir.EngineType.Activation])

            # gather buffers, partition-chunked: rank r contributes
            # [128, m*32] chunk-major; halo chunks are small rectangles
            # at rank-dependent row offsets.
            RB1, RB2 = M1 * B, M2 * B
            ag_in1 = dram.tile([KP, RB1], E8, name="ag_in1")
            ag_out1 = dram.tile([CORES * KP, RB1], E8, name="ag_out1",
                                addr_space="Shared")
            ag_in2 = dram.tile([KP, RB2], E8, name="ag_in2")
            ag_out2 = dram.tile([CORES * KP, RB2], E8, name="ag_out2",
                                addr_space="Shared")

            # --- replicated inputs ---------------------------------------
            xT_sb = persist.tile([KP, KC * B], E8)
            nc.sync.dma_start(xT_sb[:], xT_d[:])
            ident_sb = persist.tile([32, 32], BF16)
            nc.sync.dma_start(ident_sb[:], ident_d[:])
            scales_sb = persist.tile([B, 3], F32)
            nc.sync.dma_start(scales_sb[:], scales_d[:])

            a16_sb = persist.tile([B, S], BF16)
            affT_sb = persist.tile([KP, MC * B], E8)

            def xpair(pr):
                return xT_sb[:, 2 * pr * B : (2 * pr + 2) * B].rearrange(
                    "p (two b) -> p two b", two=2
                )

            # --- phase 1a: j0 slice over the wrA stream ------------------
            apA = ps.tile([B, NSA], F32, name="apA", tag="accA", bufs=1)
            aff_ps = [
                ps.tile([B, NS], F32, name=f"aff_ps{j}", tag="acc", bufs=6)
                for j in (1, 2)
            ]
            lastA = None
            for kb in range(NBLK):
                w_t = wrp.tile([KP, WBLK * NSA], E8, name="w_tA", tag="wrA")
                cs = slice(kb * WBLK * NSA, (kb + 1) * WBLK * NSA)
                lastA = nc.sync.dma_start(w_t[:], wrA_d[:, cs])
                w3 = w_t[:].rearrange("p (t s) -> p t s", s=NSA)
                for tp in range(WBLK // 2):
                    pr = kb * (WBLK // 2) + tp
                    nc.tensor.matmul(
                        apA[:, :], xpair(pr),
                        w3[:, 2 * tp : 2 * tp + 2, :],
                        start=(pr == 0), stop=(pr == PAIRS - 1),
                        perf_mode=DR,
                    )

            # piece-A tail: quantize, transpose chunks 0-3, gather #1
            nc.vector.tensor_scalar(
                a16_sb[:, 0:NSA], apA[:, :], scales_sb[:, 0:1], None,
                mybir.AluOpType.mult,
            )
            tp_ps = ps.tile([KP, MC * B], BF16, name="tp_ps", tag="tp",
                            bufs=1)
            for m in range(M1):
                nc.tensor.matmul(
                    tp_ps[:, m * B : (m + 1) * B],
                    a16_sb[:, m * KP : (m + 1) * KP],
                    ident_sb[:],
                    is_transpose=True,
                    start=(m == 0), stop=(m == M1 - 1),
                )
            nc.scalar.activation(affT_sb[:, 0 : M1 * B],
                                 tp_ps[:, 0 : M1 * B],
                                 mybir.ActivationFunctionType.Copy)
            ag1_dma = nc.scalar.dma_start(ag_in1[:], affT_sb[:, 0 : M1 * B])
            nc.gpsimd.collective_compute(
                "AllGather", mybir.AluOpType.bypass, replica_groups=rg,
                ins=[ag_in1.opt()], outs=[ag_out1.opt()],
            )

            # --- phase 1b: j1/j2 slices over the wrB stream --------------
            NSB = S - NSA
            BW = [NS, NSB - NS]  # 384, 256 column pieces of B
            lastB = None
            for kb in range(NBLK):
                w_t = wrp.tile([KP, WBLK * NSB], E8, name="w_tB",
                               tag="wrB")
                cs = slice(kb * WBLK * NSB, (kb + 1) * WBLK * NSB)
                d = nc.sync.dma_start(w_t[:], wrB_d[:, cs])
                if kb == 0:
                    add_dep_helper(d.ins, lastA.ins, sync=False,
                                   reason="wrB stream after wrA stream")
                lastB = d
                w3 = w_t[:].rearrange("p (t s) -> p t s", s=NSB)
                for tp in range(WBLK // 2):
                    pr = kb * (WBLK // 2) + tp
                    for j in (0, 1):
                        nc.tensor.matmul(
                            aff_ps[j][:, 0 : BW[j]], xpair(pr),
                            w3[:, 2 * tp : 2 * tp + 2,
                               j * NS : j * NS + BW[j]],
                            start=(pr == 0), stop=(pr == PAIRS - 1),
                            perf_mode=DR,
                        )

            # piece-B tail: quantize, transpose chunks 4-8, gather #2
            for j in (0, 1):
                js = slice(NSA + j * NS, NSA + j * NS + BW[j])
                if j == 0:
                    nc.scalar.activation(
                        a16_sb[:, js], aff_ps[j][:, 0 : BW[j]],
                        mybir.ActivationFunctionType.Copy,
                        scale=scales_sb[:, 0:1],
                    )
                else:
                    nc.vector.tensor_scalar(
                        a16_sb[:, js], aff_ps[j][:, 0 : BW[j]],
                        scales_sb[:, 0:1], None,
                        mybir.AluOpType.mult,
                    )
            for m in range(M1, MC):
                nc.tensor.matmul(
                    tp_ps[:, m * B : (m + 1) * B],
                    a16_sb[:, m * KP : (m + 1) * KP],
                    ident_sb[:],
                    is_transpose=True,
                    start=(m == M1), stop=(m == MC - 1),
                )
            nc.scalar.activation(affT_sb[:, M1 * B :], tp_ps[:, M1 * B :],
                                 mybir.ActivationFunctionType.Copy)
            ag2_dma = nc.scalar.dma_start(ag_in2[:], affT_sb[:, M1 * B :])
            ag2_cc = nc.gpsimd.collective_compute(
                "AllGather", mybir.AluOpType.bypass, replica_groups=rg,
                ins=[ag_in2.opt()], outs=[ag_out2.opt()],
            )

            # --- band + inhibitory weight streams ------------------------
            band_sb = persist.tile([KP, NJ * BANDC * NS], E8)
            band_dma = nc.scalar.dma_start(band_sb[:], band_d[:])
            add_dep_helper(band_dma.ins, ag2_dma.ins, sync=False,
                           reason="band stream behind ag_in2")
            wib_sb = persist.tile([KP, KC * S], E8)
            wib_dmas = []
            for kb in range(NWIB):
                src = slice(kb * WIBBLK * KP, (kb + 1) * WIBBLK * KP)
                d = nc.sync.dma_start(
                    wib_sb[:, kb * WIBBLK * S : (kb + 1) * WIBBLK * S]
                    .rearrange("p (t s) -> p t s", s=S),
                    wib_d[src, :].rearrange("(t p) s -> t p s", p=KP)
                    .transpose([1, 0, 2]),
                )
                if kb == 0:
                    add_dep_helper(d.ins, lastB.ins, sync=False,
                                   reason="wib stream after wr streams")
                else:
                    add_dep_helper(d.ins, wib_dmas[-1].ins, sync=False,
                                   reason="keep wib stream ordered")
                if kb == 1:
                    add_dep_helper(d.ins, ag2_dma.ins, sync=True,
                                   reason="let ag_in2 slot in early")
                wib_dmas.append(d)

            # --- band matmuls on local chunks (during the gathers) -------
            p2b = [
                ps.tile([B, NS], F32, name=f"p2b_{j}", tag="acc", bufs=6)
                for j in range(NJ)
            ]
            band3d = band_sb[:].rearrange("p (t s) -> p t s", s=NS)

            def band_mm(j, rel, nch, pos, local, start, stop):
                if local:
                    base = affT_sb[:, (pos - 2) * B : (pos - 2 + nch) * B]
                else:
                    # halo buffer: positions {0,1} at cols 0-2B, {11,12}
                    # at cols 2B-4B
                    hp = pos if pos < 2 else pos - 11 + 2
                    base = wband_sb[:, hp * B : (hp + nch) * B]
                if nch == 2:
                    nc.tensor.matmul(
                        p2b[j][:, :],
                        base.rearrange("p (two b) -> p two b", two=2),
                        band3d[:, j * BANDC + rel : j * BANDC + rel + 2, :],
                        start=start, stop=stop, perf_mode=DR,
                    )
                else:
                    nc.tensor.matmul(
                        p2b[j][:, :], base,
                        band3d[:, j * BANDC + rel, :],
                        start=start, stop=stop,
                    )

            for j in range(NJ):
                ops = [o for o in BAND_PLAN[j] if o[3]]
                for i, (rel, nch, pos, local) in enumerate(ops):
                    band_mm(j, rel, nch, pos, local,
                            start=(i == 0),
                            stop=(len([o for o in BAND_PLAN[j]
                                       if not o[3]]) == 0
                                  and i == len(ops) - 1))

            # --- unpack gathered halves + halo rectangles ----------------
            # chained with scheduling-order edges so none of their
            # collective waits head-of-line-block the Act queue's earlier
            # descriptor generation (ag_in2/band).
            affTg_sb = persist.tile([KP, KC * B], E8)
            atg = affTg_sb[:].rearrange("p (r c) -> p r c", c=MC * B)
            u1_dma = nc.scalar.dma_start(
                atg[:, :, 0:RB1],
                ag_out1[:].rearrange("(r p) c -> p r c", p=KP),
            )
            add_dep_helper(u1_dma.ins, band_dma.ins, sync=False,
                           reason="unpack 1 behind band on Act queue")
            # right halo: chunks {0,1} of rank (c+1) mod 8 (from gather 1)
            wband_sb = persist.tile([KP, 4 * B], E8)
            right_src = ag_out1[0:KP, 0 : 2 * B]
            hr_dma = nc.scalar.dma_start(
                wband_sb[:, 2 * B : 4 * B],
                AP(right_src.tensor,
                   right_src.offset + ((pid + 1) % CORES) * (KP * RB1),
                   right_src.ap,
                   dep_tracking_offset=right_src.offset),
            )
            add_dep_helper(hr_dma.ins, u1_dma.ins, sync=False,
                           reason="halo r behind unpack 1")
            u2_dma = nc.scalar.dma_start(
                atg[:, :, RB1:],
                ag_out2[:].rearrange("(r p) c -> p r c", p=KP),
            )
            add_dep_helper(u2_dma.ins, hr_dma.ins, sync=False,
                           reason="unpack 2 behind halo r")
            # left halo: chunks {7,8} of rank (c-1) mod 8 (from gather 2)
            left_src = ag_out2[0:KP, (7 - M1) * B : (9 - M1) * B]
            hl_dma = nc.scalar.dma_start(
                wband_sb[:, 0 : 2 * B],
                AP(left_src.tensor,
                   left_src.offset + ((pid + (CORES - 1)) % CORES)
                   * (KP * RB2),
                   left_src.ap,
                   dep_tracking_offset=left_src.offset),
            )
            add_dep_helper(hl_dma.ins, u2_dma.ins, sync=False,
                           reason="halo l behind unpack 2")
            add_dep_helper(wib_dmas[NWIB - 1].ins, u2_dma.ins, sync=True,
                           reason="leave room for unpack 2")

            # --- phase 2 dense: p2[j] += a8 @ (-0.4 Wi si) ---------------
            p2 = [
                ps.tile([B, NS], F32, name=f"p2_{j}", tag="acc", bufs=6)
                for j in range(NJ)
            ]
            wib3d = wib_sb[:].rearrange("p (k s) -> p k s", s=S)
            order = PASS1 + PASS2
            for idx, t2 in enumerate(order):
                lhsT = affTg_sb[:, 2 * t2 * B : (2 * t2 + 2) * B].rearrange(
                    "p (two b) -> p two b", two=2
                )
                for j in range(NJ):
                    nc.tensor.matmul(
                        p2[j][:, :],
                        lhsT,
                        wib3d[:, 2 * t2 : 2 * t2 + 2,
                              j * NS : (j + 1) * NS],
                        start=(idx == 0),
                        stop=(idx == PAIRS - 1),
                        perf_mode=DR,
                    )

            # halo-dependent band matmuls
            for j in range(NJ):
                ops = [o for o in BAND_PLAN[j] if not o[3]]
                for i, (rel, nch, pos, local) in enumerate(ops):
                    band_mm(j, rel, nch, pos, local,
                            start=False, stop=(i == len(ops) - 1))

            # --- combine: out = p2b/(s_a s_b) + p2/(s_a si) + 0.4 --------
            # (pre-activation is in [0.38, 0.42]: the clamp never binds)
            out_sb = persist.tile([B, S], F32)
            for j in range(NJ):
                js = slice(j * NS, (j + 1) * NS)
                tj = persist.tile([B, NS], F32, name=f"t0_{j}")
                nc.scalar.activation(
                    tj[:], p2b[j][:, :],
                    mybir.ActivationFunctionType.Copy,
                    bias=0.4, scale=scales_sb[:, 2:3],
                )
                nc.vector.scalar_tensor_tensor(
                    out_sb[:, js], p2[j][:, :], scales_sb[:, 1:2], tj[:],
                    mybir.AluOpType.mult, mybir.AluOpType.add,
                )
                nc.sync.dma_start(out_d[:, js], out_sb[:, js])

    nc.compile()
    return nc


_NC = None


def _get_nc():
    global _NC
    if _NC is None:
        _NC = build_nc()
    return _NC


def make_in_maps(x, retina_weights, excitatory_weights, inhibitory_weights):
    np_e8 = ml_dtypes.float8_e4m3fn

    x = np.asarray(x, dtype=np.float32)
    wr = np.asarray(retina_weights, dtype=np.float32)
    we = np.asarray(excitatory_weights, dtype=np.float32)
    wi = np.asarray(inhibitory_weights, dtype=np.float32)

    sr = 192.0 / max(float(np.abs(wr).max()), 1e-30)
    si = 192.0 / max(float(0.4 * np.abs(wi).max()), 1e-30)

    x8 = ((x - 0.5) * S_X).astype(np_e8)
    xT = np.ascontiguousarray(
        x8.reshape(B, KC, KP).transpose(2, 1, 0).reshape(KP, KC * B)
    )
    scales = np.tile(
        np.array(
            [[S_A / (S_X * sr), 1.0 / (S_A * si), 1.0 / (S_A * S_B)]],
            dtype=np.float32,
        ),
        (B, 1),
    )

    in_maps = []
    for c in range(CORES):
        sl = slice(c * S, (c + 1) * S)
        # retina slice, fp8-scaled, partition-major [128, chunk*cols]
        wr8 = (wr[:, sl] * sr).astype(np_e8)
        wr_pm = wr8.reshape(KC, KP, S).transpose(1, 0, 2)  # [128, 72, 1152]
        wrA = np.ascontiguousarray(wr_pm[:, :, 0:NSA]).reshape(KP, KC * NSA)
        wrB = np.ascontiguousarray(wr_pm[:, :, NSA:]).reshape(
            KP, KC * (S - NSA))

        band = np.zeros((NJ * BANDC * KP, NS), dtype=np.float32)
        for j in range(NJ):
            col0 = c * S + j * NS
            for t in range(BANDC):
                r0 = (9 * c + 3 * j - 2 + t) * KP
                lo, hi = max(r0, 0), min(r0 + KP, N)
                if lo < hi:
                    blk = 0.2 * we[lo:hi, col0 : col0 + NS]
                    # identity diagonal folded into the band
                    dr = np.arange(lo, hi)
                    dc = dr - col0
                    m = (dc >= 0) & (dc < NS)
                    blk[dr[m] - lo, dc[m]] += 1.0
                    band[(j * BANDC + t) * KP + (lo - r0) :
                         (j * BANDC + t) * KP + (hi - r0), :] = S_B * blk
        band_pm = np.ascontiguousarray(
            band.reshape(NJ * BANDC, KP, NS).transpose(1, 0, 2)
            .reshape(KP, NJ * BANDC * NS)
        )
        in_maps.append(
            {
                "xT": xT,
                "wrA": wrA,
                "wrB": wrB,
                "wib": (np.ascontiguousarray(wi[:, sl]) * (-0.4 * si)).astype(
                    np_e8
                ),
                "band": band_pm.astype(np_e8),
                "scales": scales,
            }
        )
    return in_maps


def _run(x, retina_weights, excitatory_weights, inhibitory_weights,
         trace=False):
    in_maps = make_in_maps(
        x, retina_weights, excitatory_weights, inhibitory_weights
    )
    res = run_bass_kernel_spmd(
        _get_nc(), in_maps, core_ids=list(range(CORES)), trace=trace
    )
    out = np.concatenate([res.results[c]["out"] for c in range(CORES)], axis=1)
    return np.ascontiguousarray(out, dtype=np.float32), res


def kernel(x, retina_weights, excitatory_weights, inhibitory_weights):
    out, _ = _run(x, retina_weights, excitatory_weights, inhibitory_weights)
    return out
